# revision 32
# baseline (speedup 1.0000x reference)
"""Softmax-gated GRU on 8 trn2 NeuronCores — striped col-tiled design.

Data-parallel over batch (bs=128 -> 16/core, weights replicated).

Phase 2 uses a "striped" layout: every H=1024-wide per-batch quantity Y
lives as Y_st[128, 256] with Y_st[32j+b, n] = Y[b, 256j+n] (4 strips of
16 batch rows at partition 32j).  Gate matmuls are emitted as 4 col-group
tiles (tile_position (0, 32j)) whose moving W slices stream concurrently
on separate XBUSes, and whose outputs land directly in the striped PSUM
layout.  All elementwise work then runs on [128, 256] tiles (4x fewer
cycles than [16, 1024]), softmax sums use accum_out + two tiny
sel/broadcast matmuls across strips, the 1/S_r scale folds into the tanh
activation scale, and the 16 per-step [16,128] transposes issue as
row-group-packed regular matmuls against a stacked identity (one PSUM
bank per row group — concurrent row groups sharing a bank wedge the
chip).  r-gate matmuls run before the z half so the r-softmax tail
overlaps the z matmuls; DMA stores use the ACT HWDGE ring while loads
use the SP ring.
"""

import sys

sys.path.insert(0, "/opt/trn_rl_repo")

import contextlib

import numpy as np
import ml_dtypes

import concourse.bass as bass
import concourse.bacc as bacc_mod
import concourse.tile as tile
from concourse import mybir
from concourse.bass_utils import run_bass_kernel_spmd

SEQ, BS, IN, H = 512, 128, 512, 1024
NCORES = 8
BSC = BS // NCORES          # 16 batch rows per core
KCH = H // 128              # 8 contraction chunks (h part)
KXC = IN // 128             # 4 contraction chunks (x part)
NTOT = 3 * H                # gates [z, r, h] concatenated: 3072
NSL = 512                   # phase-1 slice
NST = 256                   # strip width (H / 4)
F32, BF16 = mybir.dt.float32, mybir.dt.bfloat16
AF = mybir.ActivationFunctionType

PACK_TRANS = True           # 4-way row-group packing for PE transposes


def build_nc(seq=SEQ):
    nc = bacc_mod.Bacc()
    xT = nc.declare_dram_parameter("xT", [KXC, 128, seq, BSC], BF16, isOutput=False)
    h0st = nc.declare_dram_parameter("h0st", [128, NST], F32, isOutput=False)
    # wzr[p, k, j, 0:256]=Wz_h slice j, [256:512]=Wr_h slice j
    wzr = nc.declare_dram_parameter("wzr", [128, KCH, 4, 2 * NST], BF16, isOutput=False)
    wh4 = nc.declare_dram_parameter("wh4", [128, KCH, 4, NST], BF16, isOutput=False)
    wxx = nc.declare_dram_parameter("wxx", [KXC, 128, NTOT], BF16, isOutput=False)
    bias = nc.declare_dram_parameter("bias", [1, NTOT], BF16, isOutput=False)
    e4 = nc.declare_dram_parameter("e4", [128, BSC], BF16, isOutput=False)
    sel4 = nc.declare_dram_parameter("sel4", [128, BSC], F32, isOutput=False)
    bc4 = nc.declare_dram_parameter("bc4", [BSC, 128], F32, isOutput=False)
    ones1 = nc.declare_dram_parameter("ones1", [1, 128], BF16, isOutput=False)
    out = nc.declare_dram_parameter("out", [seq, 128, NST], BF16, isOutput=True)
    # xdram[t, j, u, (g n)]: strip j's free layout is [z_j | r_j | h_j]
    xdram = nc.dram_tensor("xscratch", [seq, 4, BSC, NTOT // 4], BF16)

    n_mt = seq * BSC // 128  # phase-1 M-tiles (8 steps each)

    with tile.TileContext(nc) as tc:
        with contextlib.ExitStack() as ctx:
            consts = ctx.enter_context(tc.tile_pool(name="consts", bufs=1))
            wpool = ctx.enter_context(tc.tile_pool(name="w", bufs=1))
            xtp = ctx.enter_context(tc.tile_pool(name="xt", bufs=3))
            stg = ctx.enter_context(tc.tile_pool(name="stg", bufs=3))
            xs = ctx.enter_context(tc.tile_pool(name="xs", bufs=4))
            hpool = ctx.enter_context(tc.tile_pool(name="h", bufs=3))
            tails = ctx.enter_context(tc.tile_pool(name="tails", bufs=3))
            scal = ctx.enter_context(tc.tile_pool(name="scal", bufs=3))
            ps_zr = ctx.enter_context(tc.tile_pool(name="ps_zr", bufs=1, space="PSUM"))
            ps_h = ctx.enter_context(tc.tile_pool(name="ps_h", bufs=1, space="PSUM"))
            ps_tr = [
                ctx.enter_context(
                    tc.tile_pool(name=f"ps_t{j}", bufs=1, space="PSUM")
                )
                for j in range(4)
            ]
            ps_sm = ps_tr[0]
            ps_x = ctx.enter_context(tc.tile_pool(name="ps_x", bufs=1, space="PSUM"))

            wzr_sb = wpool.tile([128, KCH, 4, 2 * NST], BF16)
            nc.sync.dma_start(wzr_sb[:], wzr[:])
            wh_sb = wpool.tile([128, KCH, 4, NST], BF16)
            nc.sync.dma_start(wh_sb[:], wh4[:])
            wx_sb = wpool.tile([128, KXC, NTOT], BF16)
            nc.sync.dma_start(wx_sb[:], wxx.rearrange("k p n -> p k n"))
            b_sb = consts.tile([1, NTOT], BF16)
            nc.sync.dma_start(b_sb[:], bias[:])
            e4_sb = consts.tile([128, BSC], BF16)
            nc.sync.dma_start(e4_sb[:], e4[:])
            sel4_sb = consts.tile([128, BSC], F32)
            nc.sync.dma_start(sel4_sb[:], sel4[:])
            bc4_sb = consts.tile([BSC, 128], F32)
            nc.sync.dma_start(bc4_sb[:], bc4[:])
            on_sb = consts.tile([1, 128], BF16)
            nc.sync.dma_start(on_sb[:], ones1[:])

            # ------- phase 1 (lazy): X = x @ W_x + b -> DRAM, striped -------
            NPS = NTOT // NSL  # 6 slices

            def stage_out(mt, stage):
                # stage free layout is (j, g, n); one DMA per strip j
                for j in range(4):
                    nc.scalar.dma_start(
                        xdram[mt * 8 : (mt + 1) * 8, j],
                        stage[:, j * (NTOT // 4) : (j + 1) * (NTOT // 4)],
                    )



            def emit_mtile(mt):
                xt_sb = xtp.tile([128, KXC, 128], BF16, tag="xt")
                nc.sync.dma_start(
                    xt_sb[:],
                    xT[:, :, mt * 8 : (mt + 1) * 8, :]
                    .rearrange("k p t b -> p k (t b)"),
                )
                stage = stg.tile([128, NTOT], BF16, tag="stage")
                for s in range(NPS):
                    px = ps_x.tile([128, NSL], F32, tag="px")
                    nc.tensor.matmul(
                        px[:], on_sb[:], b_sb[:, s * NSL : (s + 1) * NSL],
                        start=True, stop=False,
                    )
                    for k in range(KXC):
                        nc.tensor.matmul(
                            px[:], xt_sb[:, k, :],
                            wx_sb[:, k, s * NSL : (s + 1) * NSL],
                            start=False, stop=(k == KXC - 1),
                        )
                    if s % 2:
                        nc.scalar.copy(stage[:, s * NSL : (s + 1) * NSL], px[:])
                    else:
                        nc.vector.tensor_copy(stage[:, s * NSL : (s + 1) * NSL], px[:])
                stage_out(mt, stage)

            P1_LEAD = 5  # m-tiles emitted ahead of the consuming step
            for mt in range(min(P1_LEAD, n_mt)):
                emit_mtile(mt)

            p1 = {"mt": P1_LEAD, "s": 0, "xt": None, "stage": None}

            def emit_slice():
                if p1["mt"] >= n_mt:
                    return
                mt, s = p1["mt"], p1["s"]
                if s == 0:
                    p1["xt"] = xtp.tile([128, KXC, 128], BF16, tag="xt", name="p1xt")
                    nc.sync.dma_start(
                        p1["xt"][:],
                        xT[:, :, mt * 8 : (mt + 1) * 8, :]
                        .rearrange("k p t b -> p k (t b)"),
                    )
                    p1["stage"] = stg.tile([128, NTOT], BF16, tag="stage", name="p1stage")
                xt_sb, stage = p1["xt"], p1["stage"]
                px = ps_x.tile([128, NSL], F32, tag="px")
                nc.tensor.matmul(
                    px[:], on_sb[:], b_sb[:, s * NSL : (s + 1) * NSL],
                    start=True, stop=False,
                )
                for k in range(KXC):
                    nc.tensor.matmul(
                        px[:], xt_sb[:, k, :],
                        wx_sb[:, k, s * NSL : (s + 1) * NSL],
                        start=False, stop=(k == KXC - 1),
                    )
                if s % 2:
                    nc.scalar.copy(stage[:, s * NSL : (s + 1) * NSL], px[:])
                else:
                    nc.vector.tensor_copy(stage[:, s * NSL : (s + 1) * NSL], px[:])
                if s == NPS - 1:
                    stage_out(mt, stage)
                    p1["mt"], p1["s"] = mt + 1, 0
                else:
                    p1["s"] = s + 1

            # ---------------- phase 2: recurrence (striped) ----------------
            def transpose_to(dst_sb, src_st, tag):
                """src_st [128, 256] striped -> dst_sb [128, 128] transposed
                (chunk c at cols 16c), via regular matmuls against the
                stacked identity.  Row-group-concurrent tiles need distinct
                PSUM banks, hence one pool per strip j."""
                trps = [
                    ps_tr[j].tile(
                        [128, 2 * BSC], F32, tag=f"trp{j}", name=f"trp{j}_{tag}"
                    )
                    for j in range(4)
                ]
                for half in range(2):
                    for j in range(4):
                        nc.tensor.matmul(
                            trps[j][:, half * BSC : (half + 1) * BSC],
                            src_st[32 * j : 32 * j + BSC,
                                   half * 128 : (half + 1) * 128],
                            e4_sb[32 * j : 32 * j + BSC, :],
                            start=True, stop=True,
                            tile_position=(32 * j, 0),
                            skip_group_check=True,
                        )
                for j in range(4):
                    eng = nc.vector.tensor_copy if j % 2 == 0 else nc.scalar.copy
                    eng(dst_sb[:, 32 * j : 32 * j + 2 * BSC], trps[j][:])

            h_st = hpool.tile([128, NST], F32, tag="h_st")
            nc.sync.dma_start(h_st[:], h0st[:])
            h_bf = hpool.tile([128, NST], BF16, tag="h_bf")
            nc.vector.tensor_copy(h_bf[:], h_st[:])
            hT = hpool.tile([128, 128], BF16, tag="hT")
            transpose_to(hT, h_bf, "h0")


            def load_xx(t):
                x4 = xs.tile([128, NTOT // 4], BF16, tag="xx")
                for j in range(4):
                    nc.sync.dma_start(
                        x4[32 * j : 32 * j + BSC, :], xdram[t, j]
                    )
                return x4

            xx_q = {}
            for tt in range(min(3, seq)):
                xx_q[tt] = load_xx(tt)

            # gate psums are allocated once (memset keeps the unused strip
            # lanes finite for the full-width exp reads)
            pzr = ps_zr.tile([128, 2 * NST], F32, tag="pzr")
            nc.vector.memset(pzr[:], 0.0)
            ph = ps_h.tile([128, NST], F32, tag="ph")
            nc.vector.memset(ph[:], 0.0)

            for t in range(seq):
                if t + 3 < seq:
                    xx_q[t + 3] = load_xx(t + 3)
                xx = xx_q.pop(t)

                # injects: diagonal col-tiles, start the accumulations
                for j in range(4):
                    nc.tensor.matmul(
                        pzr[32 * j : 32 * j + BSC, :],
                        e4_sb[32 * j : 32 * j + BSC, :],
                        xx[32 * j : 32 * j + BSC, 0 : 2 * NST],
                        start=True, stop=False,
                        tile_position=(32 * j, 32 * j),
                        skip_group_check=True,
                    )
                for j in range(4):
                    nc.tensor.matmul(
                        ph[32 * j : 32 * j + BSC, :],
                        e4_sb[32 * j : 32 * j + BSC, :],
                        xx[32 * j : 32 * j + BSC, 2 * NST : 3 * NST],
                        start=True, stop=False,
                        tile_position=(32 * j, 32 * j),
                        skip_group_check=True,
                    )

                # r gate matmuls first: per k, 4 concurrent col-group tiles
                for k in range(KCH):
                    for j in range(4):
                        nc.tensor.matmul(
                            pzr[32 * j : 32 * j + BSC, NST : 2 * NST],
                            hT[:, k * BSC : (k + 1) * BSC],
                            wzr_sb[:, k, j, NST : 2 * NST],
                            start=False, stop=(k == KCH - 1),
                            tile_position=(0, 32 * j),
                            skip_group_check=True,
                        )

                # r softmax pieces (r in cols 256:512)
                er = tails.tile([128, NST], BF16, tag="er")
                rsum = scal.tile([128, 1], F32, tag="rsum")
                nc.scalar.activation(
                    er[:], pzr[:, NST : 2 * NST], AF.Exp, accum_out=rsum[:]
                )
                psm = ps_sm.tile([128, 4], F32, tag="psm")
                nc.tensor.matmul(
                    psm[0:BSC, 0:1], sel4_sb[:], rsum[:], start=True, stop=True,
                    skip_group_check=True,
                )
                sr16 = scal.tile([BSC, 1], F32, tag="sr16")
                nc.vector.reciprocal(sr16[:], psm[0:BSC, 0:1])

                # z gate matmuls (overlap the r softmax tail on ACT/DVE)
                for k in range(KCH):
                    for j in range(4):
                        nc.tensor.matmul(
                            pzr[32 * j : 32 * j + BSC, 0:NST],
                            hT[:, k * BSC : (k + 1) * BSC],
                            wzr_sb[:, k, j, 0:NST],
                            start=False, stop=(k == KCH - 1),
                            tile_position=(0, 32 * j),
                            skip_group_check=True,
                        )

                # z softmax exp early (ACT op only - no PE-queue coupling)
                ez = tails.tile([128, NST], BF16, tag="ez")
                zsum = scal.tile([128, 1], F32, tag="zsum")
                nc.scalar.activation(ez[:], pzr[:, 0:NST], AF.Exp, accum_out=zsum[:])

                # u = er (.) h  (1/S_r folds into the tanh scale later)
                u_st = tails.tile([128, NST], BF16, tag="u_st")
                nc.vector.tensor_mul(u_st[:], er[:], h_bf[:])
                uT = hpool.tile([128, 128], BF16, tag="uT")
                transpose_to(uT, u_st, "u")

                # candidate matmuls; the 1/S_r broadcast slots in after
                # k=0 (recip is long done) so tanh unblocks at stream end
                def cand_k(k):
                    for j in range(4):
                        nc.tensor.matmul(
                            ph[32 * j : 32 * j + BSC, :],
                            uT[:, k * BSC : (k + 1) * BSC],
                            wh_sb[:, k, j, :],
                            start=False, stop=(k == KCH - 1),
                            tile_position=(0, 32 * j),
                            skip_group_check=True,
                        )

                cand_k(0)
                nc.tensor.matmul(
                    psm[:, 1:2], bc4_sb[:], sr16[:], start=True, stop=True,
                    skip_group_check=True,
                )
                sr_st = scal.tile([128, 1], F32, tag="sr_st")
                nc.vector.tensor_copy(sr_st[:], psm[:, 1:2])
                for k in range(1, KCH):
                    cand_k(k)
                nc.tensor.matmul(
                    psm[0:BSC, 2:3], sel4_sb[:], zsum[:], start=True, stop=True,
                    skip_group_check=True,
                )
                sz16 = scal.tile([BSC, 1], F32, tag="sz16")
                nc.vector.reciprocal(sz16[:], psm[0:BSC, 2:3])
                nc.tensor.matmul(
                    psm[:, 3:4], bc4_sb[:], sz16[:], start=True, stop=True,
                    skip_group_check=True,
                )
                ezn = tails.tile([128, NST], BF16, tag="ezn")
                nc.vector.tensor_scalar_mul(ezn[:], ez[:], psm[:, 3:4])

                # tail: th = tanh(ph/S_r); h' = h + ezn (.) (th - h)
                th = tails.tile([128, NST], BF16, tag="th")
                nc.scalar.activation(th[:], ph[:], AF.Tanh, scale=sr_st[:])
                mz = tails.tile([128, NST], BF16, tag="mz")
                nc.vector.tensor_sub(mz[:], th[:], h_bf[:])
                nc.vector.tensor_mul(mz[:], ezn[:], mz[:])
                h_new = hpool.tile([128, NST], F32, tag="h_st")
                nc.vector.tensor_add(h_new[:], h_st[:], mz[:])
                h_bf_new = hpool.tile([128, NST], BF16, tag="h_bf")
                nc.vector.tensor_copy(h_bf_new[:], h_new[:])
                nc.scalar.dma_start(out[t], h_bf_new[:])
                if t + 1 < seq:
                    hT_new = hpool.tile([128, 128], BF16, tag="hT")
                    transpose_to(hT_new, h_bf_new, "h")
                    hT = hT_new
                h_st = h_new
                h_bf = h_bf_new
                # phase-1 slice at the step's end: the in-order PE drains
                # the chain-critical transposes first, then fills the gap
                # until the next step's r-matmuls with phase-1 work
                emit_slice()
    nc.compile()
    return nc


def prep_inputs(x, h0, Wz, bz, Wr, br, Wh, bh, seq=SEQ):
    bf = ml_dtypes.bfloat16
    Wzh, Wrh, Whh = (np.asarray(w[:H], np.float32) for w in (Wz, Wr, Wh))
    wxx = np.concatenate([Wz[H:], Wr[H:], Wh[H:]], axis=1)   # [512, 3072]
    bias = np.concatenate([bz, br, bh])[None, :]
    # reorder x-path columns to the striped (j, g, n) layout
    perm = np.array(
        [g * H + j * NST + n
         for j in range(4) for g in range(3) for n in range(NST)]
    )
    wxx = wxx[:, perm]
    bias = bias[:, perm]

    def strip_w(w):  # [1024, 1024] -> [128p, 8k, 4j, 256n]
        return np.ascontiguousarray(
            w.reshape(KCH, 128, 4, NST).transpose(1, 0, 2, 3)
        )

    wzr = np.concatenate([strip_w(Wzh), strip_w(Wrh)], axis=3)  # [...,512]
    wh4 = strip_w(Whh)

    e4 = np.zeros((128, BSC), np.float32)
    bc4 = np.zeros((BSC, 128), np.float32)
    for j in range(4):
        for b in range(BSC):
            e4[32 * j + b, b] = 1.0
            bc4[b, 32 * j + b] = 1.0

    shared = {
        "wzr": wzr.astype(bf),
        "wh4": wh4.astype(bf),
        "wxx": np.ascontiguousarray(wxx.reshape(KXC, 128, NTOT)).astype(bf),
        "bias": bias.astype(bf),
        "e4": e4.astype(bf),
        "sel4": e4.astype(np.float32),
        "bc4": bc4.astype(np.float32),
        "ones1": np.ones((1, 128), dtype=bf),
    }
    maps = []
    for c in range(NCORES):
        sl = slice(c * BSC, (c + 1) * BSC)
        xc = np.asarray(x[:seq, sl, :], dtype=np.float32)
        xTc = np.ascontiguousarray(xc.transpose(2, 0, 1)).reshape(
            KXC, 128, seq, BSC
        )
        h0c = np.asarray(h0[sl], np.float32)  # [16, 1024]
        h0st = np.zeros((128, NST), np.float32)
        h0r = h0c.reshape(BSC, 4, NST)
        for j in range(4):
            h0st[32 * j : 32 * j + BSC, :] = h0r[:, j, :]
        maps.append(dict(shared, xT=xTc.astype(bf), h0st=h0st))
    return maps


def unstripe(o, seq=SEQ):
    """[seq, 128, 256] striped (+gap rows) -> [seq, 16, 1024] f32."""
    o = np.asarray(o, dtype=np.float32).reshape(seq, 4, 32, NST)[:, :, :BSC, :]
    return np.ascontiguousarray(o.transpose(0, 2, 1, 3)).reshape(seq, BSC, H)


LAST_EXEC_NS = None


def kernel(x, h0, Wz, bz, Wr, br, Wh, bh):
    global LAST_EXEC_NS
    nc = build_nc(SEQ)
    maps = prep_inputs(x, h0, Wz, bz, Wr, br, Wh, bh, SEQ)
    res = run_bass_kernel_spmd(nc, maps, list(range(NCORES)))
    if res.exec_time_ns is not None:
        LAST_EXEC_NS = res.exec_time_ns
    outs = [unstripe(res.results[c]["out"]) for c in range(NCORES)]
    return np.concatenate(outs, axis=1).astype(np.float32)
